# revision 6
# baseline (speedup 1.0000x reference)
"""Trainium2 Bass kernel for a 3-relation heterogeneous GraphConv encoder.

Graph: papers (50000) and authors (20000), D=H=128.
  out_paper  = mean_writes @ Wr_w.T + br_w + mean_cites @ Wr_c.T + br_c
               + x_paper @ (Wo_w + Wo_c).T
  out_author = mean_written @ Wr_n.T + br_n + x_author @ Wo_n.T
where mean_* is the per-destination mean over incoming edges of source features.

Sharding: destination nodes (and their incoming edges) are partitioned across
8 cores (papers 6250/core, authors 2500/core). Source feature tables are
replicated (bf16) and gathered per edge with dma_gather. Edges are bucketed by
destination tile (128 dsts) on the host; each run is padded to a common length
so all 8 cores execute one SPMD program. Segment sums run on the TensorEngine
as one-hot matmuls (P[e, d] one-hot of in-tile dst), accumulated in PSUM fp32.
Means are divided by host-provided 1/clip(cnt,1), transposed on the PE, and
pushed through the (host-pre-transposed) relation weights; outputs are written
feature-major and transposed back on the host.
"""

import numpy as np
import ml_dtypes

import concourse.bacc as bacc
import concourse.mybir as mybir
import concourse.tile as tile
from concourse.bass_utils import run_bass_kernel_spmd
from concourse.library_config import mlp
from concourse.masks import make_identity

N_PAPER, N_AUTHOR, D = 50000, 20000, 128
NCORES = 8
PCHUNK, ACHUNK = N_PAPER // NCORES, N_AUTHOR // NCORES  # 6250, 2500
PT = (PCHUNK + 127) // 128   # 49 paper dst tiles / core
AT = (ACHUNK + 127) // 128   # 20 author dst tiles / core
HALF = 25000                 # paper gather table split (int16 index range)
G = 8192                     # gather chunk size (edges)
PB = 8                       # P-build batch (blocks)

BF16 = ml_dtypes.bfloat16

# stream key -> (n_dst_tiles, table key)
STREAMS = {
    "w":  (PT, "tab_a"),    # writes:   author -> paper
    "cA": (PT, "tab_pA"),   # cites:    paper(<25000) -> paper
    "cB": (PT, "tab_pB"),   # cites:    paper(>=25000) -> paper
    "nA": (AT, "tab_pA"),   # written:  paper(<25000) -> author
    "nB": (AT, "tab_pB"),   # written:  paper(>=25000) -> author
}


def _bucket(src, dst, chunk, c):
    """Select edges into core c's dst chunk; return (src, tile_idx, q) sorted by tile."""
    lo, hi = c * chunk, (c + 1) * chunk
    sel = (dst >= lo) & (dst < hi)
    s = src[sel]
    loc = dst[sel] - lo
    t = loc >> 7
    order = np.argsort(t, kind="stable")
    return s[order], t[order], (loc & 127)[order]


def _prep_streams(inputs):
    """Host preprocessing: per-core bucketed/padded edge streams + counts."""
    ei = {
        "w": (np.asarray(inputs["ei_writes_src"]), np.asarray(inputs["ei_writes_dst"]), PCHUNK, PT),
        "c": (np.asarray(inputs["ei_cites_src"]), np.asarray(inputs["ei_cites_dst"]), PCHUNK, PT),
        "n": (np.asarray(inputs["ei_written_src"]), np.asarray(inputs["ei_written_dst"]), ACHUNK, AT),
    }
    # per (core, stream): (src_sorted, tile_sorted, q_sorted), and per-core dst counts
    raw = {k: [] for k in STREAMS}
    cnts = {"p_w": [], "p_c": [], "a_n": []}
    for c in range(NCORES):
        for rel, (src, dst, chunk, nt) in ei.items():
            s, t, q = _bucket(src, dst, chunk, c)
            cnt = np.bincount(t * 128 + q, minlength=nt * 128).astype(np.float32)
            if rel == "w":
                raw["w"].append((s, t, q))
                cnts["p_w"].append(cnt)
            else:
                a = s < HALF
                kA, kB = ("cA", "cB") if rel == "c" else ("nA", "nB")
                raw[kA].append((s[a], t[a], q[a]))
                raw[kB].append((s[~a] - HALF, t[~a], q[~a]))
                cnts["p_c" if rel == "c" else "a_n"].append(cnt)

    # common padded run length per stream (multiple of 128, same on all cores)
    nbs = {}
    for k, (nt, _) in STREAMS.items():
        mx = 1
        for c in range(NCORES):
            _, t, _ = raw[k][c]
            mx = max(mx, int(np.bincount(t, minlength=nt).max()))
        nbs[k] = (mx + 127) // 128

    # build padded idx/q arrays per core
    per_core = [dict() for _ in range(NCORES)]
    for k, (nt, _) in STREAMS.items():
        L = nbs[k] * 128
        S = nt * L
        for c in range(NCORES):
            s, t, q = raw[k][c]
            idx_flat = np.zeros(S, np.int16)
            q_flat = np.full(S, -1.0, np.float32)
            runs = np.bincount(t, minlength=nt)
            off = np.concatenate([[0], np.cumsum(runs)[:-1]])
            pos = np.arange(len(t)) - off[t] + t * L
            idx_flat[pos] = s.astype(np.int16)
            q_flat[pos] = q
            # idx: wrapped [16, S/16] -> replicated to 128 partitions
            wrapped = idx_flat.reshape(-1, 16).T
            per_core[c][f"idx_{k}"] = np.ascontiguousarray(np.tile(wrapped, (8, 1)))
            # q: [128, S/128], col = block, row = edge-within-block
            per_core[c][f"q_{k}"] = np.ascontiguousarray(
                q_flat.reshape(-1, 128).T.astype(BF16))

    # inverse counts [128, nt] with [d, j] = 1/clip(cnt[j*128+d], 1)
    for c in range(NCORES):
        for key, nt, name in (("p_w", PT, "ic_pw"), ("p_c", PT, "ic_pc"), ("a_n", AT, "ic_an")):
            ic = 1.0 / np.clip(cnts[key][c], 1.0, None)
            per_core[c][name] = np.ascontiguousarray(ic.reshape(nt, 128).T)
    return nbs, per_core


def _build_program(nbs, pt_lim=PT, at_lim=AT):
    nc = bacc.Bacc("TRN2", target_bir_lowering=False)
    f32, bf16, i16 = mybir.dt.float32, mybir.dt.bfloat16, mybir.dt.int16

    tabs = {
        "tab_a": nc.dram_tensor("tab_a", [N_AUTHOR, D], bf16, kind="ExternalInput"),
        "tab_p": nc.dram_tensor("tab_p", [N_PAPER, D], bf16, kind="ExternalInput"),
    }
    idx_d, q_d = {}, {}
    for k, (nt, _) in STREAMS.items():
        S = nt * nbs[k] * 128
        idx_d[k] = nc.dram_tensor(f"idx_{k}", [128, S // 16], i16, kind="ExternalInput")
        q_d[k] = nc.dram_tensor(f"q_{k}", [128, S // 128], bf16, kind="ExternalInput")
    ic_pw = nc.dram_tensor("ic_pw", [128, PT], f32, kind="ExternalInput")
    ic_pc = nc.dram_tensor("ic_pc", [128, PT], f32, kind="ExternalInput")
    ic_an = nc.dram_tensor("ic_an", [128, AT], f32, kind="ExternalInput")
    xT_p = nc.dram_tensor("xT_p", [128, PT * 128], f32, kind="ExternalInput")
    xT_a = nc.dram_tensor("xT_a", [128, AT * 128], f32, kind="ExternalInput")
    wd = {n: nc.dram_tensor(n, [128, 128], f32, kind="ExternalInput")
          for n in ("wrT_w", "wrT_c", "wrT_n", "woT_w", "woT_c", "woT_n")}
    brd = {n: nc.dram_tensor(n, [128, 1], f32, kind="ExternalInput")
           for n in ("br_w", "br_c", "br_n")}
    iota_d = nc.dram_tensor("iota", [128, PB * 128], bf16, kind="ExternalInput")
    outT_p = nc.dram_tensor("outT_p", [128, PT * 128], f32, kind="ExternalOutput")
    outT_a = nc.dram_tensor("outT_a", [128, AT * 128], f32, kind="ExternalOutput")

    # table AP per stream
    def table_ap(k):
        t = STREAMS[k][1]
        if t == "tab_a":
            return tabs["tab_a"][:, :]
        if t == "tab_pA":
            return tabs["tab_p"][0:HALF, :]
        return tabs["tab_p"][HALF:, :]

    with tile.TileContext(nc) as tc:
        nc.gpsimd.load_library(mlp)
        import contextlib
        with contextlib.ExitStack() as ctx:
            const = ctx.enter_context(tc.tile_pool(name="const", bufs=1))
            idxp = ctx.enter_context(tc.tile_pool(name="idxp", bufs=4))
            chunkp = ctx.enter_context(tc.tile_pool(name="chunkp", bufs=5))
            pp = ctx.enter_context(tc.tile_pool(name="pp", bufs=4))
            mp = ctx.enter_context(tc.tile_pool(name="mp", bufs=3))
            op = ctx.enter_context(tc.tile_pool(name="op", bufs=2))
            ps_seg = ctx.enter_context(tc.tile_pool(name="ps_seg", bufs=4, space="PSUM"))
            ps_tr = ctx.enter_context(tc.tile_pool(name="ps_tr", bufs=2, space="PSUM"))
            ps_fin = ctx.enter_context(tc.tile_pool(name="ps_fin", bufs=2, space="PSUM"))

            # ---- constants ----
            iota_t = const.tile([128, PB * 128], bf16)
            nc.sync.dma_start(iota_t[:], iota_d[:])
            ic_t = {}
            for name, dram, nt in (("pw", ic_pw, PT), ("pc", ic_pc, PT), ("an", ic_an, AT)):
                t = const.tile([128, nt], f32, tag=f"ic_{name}")
                nc.sync.dma_start(t[:], dram[:])
                ic_t[name] = t
            xT_pt = const.tile([128, PT * 128], f32, tag="xtp")
            nc.sync.dma_start(xT_pt[:], xT_p[:])
            xT_at = const.tile([128, AT * 128], f32, tag="xta")
            nc.sync.dma_start(xT_at[:], xT_a[:])
            w_t = {}
            for n in wd:
                t = const.tile([128, 128], f32, tag=n)
                nc.sync.dma_start(t[:], wd[n][:])
                w_t[n] = t
            br_t = {}
            for n in brd:
                t = const.tile([128, 1], f32, tag=n)
                nc.sync.dma_start(t[:], brd[n][:])
                br_t[n] = t
            ident = const.tile([128, 128], f32, tag="ident")
            make_identity(nc, ident[:])
            woT_p = const.tile([128, 128], f32, tag="woT_p")
            nc.vector.tensor_add(woT_p[:], w_t["woT_w"][:], w_t["woT_c"][:])
            br_p = const.tile([128, 1], f32, tag="br_p")
            nc.vector.tensor_add(br_p[:], br_t["br_w"][:], br_t["br_c"][:])
            # q arrays resident in SBUF
            q_t = {}
            for k, (nt, _) in STREAMS.items():
                S = nt * nbs[k] * 128
                t = const.tile([128, S // 128], bf16, tag=f"q_{k}")
                nc.sync.dma_start(t[:], q_d[k][:])
                q_t[k] = t

            # ---- gather machinery ----
            chunks = {}  # (stream, chunk_idx) -> (tile, blocks_in_chunk)

            def get_block(k, g):
                nt, _ = STREAMS[k]
                S = nt * nbs[k] * 128
                ci = g // (G // 128)
                if (k, ci) not in chunks:
                    sz = min(G, S - ci * G)
                    nbk = sz // 128
                    it = idxp.tile([128, sz // 16], i16, tag="idx")
                    nc.sync.dma_start(it[:], idx_d[k][:, ci * (G // 16): ci * (G // 16) + sz // 16])
                    ct = chunkp.tile([128, nbk, 128], bf16, tag="chunk")
                    nc.gpsimd.dma_gather(ct[:], table_ap(k), it[:], sz, sz, D, single_packet=False)
                    chunks[(k, ci)] = ct
                return chunks[(k, ci)][:, g % (G // 128), :]

            def build_p(k, j, t0, bw):
                """P tile [128, bw, 128] for blocks t0..t0+bw of dst tile j."""
                nb = nbs[k]
                pt_ = pp.tile([128, bw, 128], bf16, tag="p")
                sl = q_t[k][:, j * nb + t0: j * nb + t0 + bw]
                nc.vector.tensor_tensor(
                    out=pt_[:],
                    in0=iota_t[:, : bw * 128].rearrange("p (a b) -> p a b", b=128),
                    in1=sl[:, :, None].to_broadcast([128, bw, 128]),
                    op=mybir.AluOpType.is_equal,
                )
                return pt_

            def seg_accum(ps, specs):
                """Accumulate one-hot matmuls into psum `ps`.
                specs: list of (stream, dst_tile_j)."""
                total = sum(nbs[k] for k, _ in specs)
                done = 0
                for k, j in specs:
                    nb = nbs[k]
                    for t0 in range(0, nb, PB):
                        bw = min(PB, nb - t0)
                        pt_ = build_p(k, j, t0, bw)
                        for b in range(bw):
                            g = j * nb + t0 + b
                            nc.tensor.matmul(
                                ps[:],
                                lhsT=pt_[:, b, :],
                                rhs=get_block(k, g),
                                start=(done == 0),
                                stop=(done == total - 1),
                            )
                            done += 1

            def mean_T(ps, ic_tile, j):
                """psum segment sums -> mean (SBUF) -> transposed mean (SBUF)."""
                m = mp.tile([128, 128], f32, tag="mean")
                nc.vector.tensor_scalar_mul(m[:], ps[:], ic_tile[:, j:j + 1])
                ptr = ps_tr.tile([128, 128], f32, tag="tr", space="PSUM")
                nc.tensor.transpose(ptr[:], m[:], ident[:])
                mt = mp.tile([128, 128], f32, tag="meanT")
                nc.scalar.copy(mt[:], ptr[:])
                return mt

            # ---- paper dst tiles ----
            stage = None
            for j in range(pt_lim):
                if j % 8 == 0:
                    gw = min(8, pt_lim - j)
                    stage = op.tile([128, gw * 128], f32, tag="stage_p")
                psw = ps_seg.tile([128, 128], f32, tag="seg", space="PSUM")
                seg_accum(psw, [("w", j)])
                psc = ps_seg.tile([128, 128], f32, tag="seg", space="PSUM")
                seg_accum(psc, [("cA", j), ("cB", j)])
                mtw = mean_T(psw, ic_t["pw"], j)
                mtc = mean_T(psc, ic_t["pc"], j)
                po = ps_fin.tile([128, 128], f32, tag="fin", space="PSUM")
                nc.tensor.matmul(po[:], lhsT=w_t["wrT_w"][:], rhs=mtw[:], start=True, stop=False)
                nc.tensor.matmul(po[:], lhsT=w_t["wrT_c"][:], rhs=mtc[:], start=False, stop=False)
                nc.tensor.matmul(po[:], lhsT=woT_p[:], rhs=xT_pt[:, j * 128:(j + 1) * 128],
                                 start=False, stop=True)
                nc.scalar.activation(stage[:, (j % 8) * 128:(j % 8) * 128 + 128], po[:],
                                     mybir.ActivationFunctionType.Identity, bias=br_p[:, :1])
                if j % 8 == 7 or j == pt_lim - 1:
                    j0 = (j // 8) * 8
                    nc.sync.dma_start(outT_p[:, j0 * 128:(j + 1) * 128], stage[:])

            # ---- author dst tiles ----
            for j in range(at_lim):
                if j % 8 == 0:
                    gw = min(8, at_lim - j)
                    stage = op.tile([128, gw * 128], f32, tag="stage_a")
                psn = ps_seg.tile([128, 128], f32, tag="seg", space="PSUM")
                seg_accum(psn, [("nA", j), ("nB", j)])
                mtn = mean_T(psn, ic_t["an"], j)
                po = ps_fin.tile([128, 128], f32, tag="fin", space="PSUM")
                nc.tensor.matmul(po[:], lhsT=w_t["wrT_n"][:], rhs=mtn[:], start=True, stop=False)
                nc.tensor.matmul(po[:], lhsT=w_t["woT_n"][:], rhs=xT_at[:, j * 128:(j + 1) * 128],
                                 start=False, stop=True)
                nc.scalar.activation(stage[:, (j % 8) * 128:(j % 8) * 128 + 128], po[:],
                                     mybir.ActivationFunctionType.Identity, bias=br_t["br_n"][:, :1])
                if j % 8 == 7 or j == at_lim - 1:
                    j0 = (j // 8) * 8
                    nc.sync.dma_start(outT_a[:, j0 * 128:(j + 1) * 128], stage[:])

    nc.compile()
    return nc


def _run(inputs, trace=False):
    inputs = {k: np.asarray(v) for k, v in inputs.items()}
    nbs, per_core = _prep_streams(inputs)
    nc = _build_program(nbs)

    tab_a = inputs["x_author"].astype(BF16)
    tab_p = inputs["x_paper"].astype(BF16)
    iota = np.ascontiguousarray(
        np.broadcast_to(np.tile(np.arange(128, dtype=np.float32), PB), (128, PB * 128))
    ).astype(BF16)

    # per-relation weights: host provides transposed layouts (layout prep only)
    wmap = {
        "wrT_w": inputs["Wr_writes"].T, "wrT_c": inputs["Wr_cites"].T, "wrT_n": inputs["Wr_written"].T,
        "woT_w": inputs["Wo_writes"].T, "woT_c": inputs["Wo_cites"].T, "woT_n": inputs["Wo_written"].T,
    }
    bmap = {
        "br_w": inputs["br_writes"].reshape(128, 1), "br_c": inputs["br_cites"].reshape(128, 1),
        "br_n": inputs["br_written"].reshape(128, 1),
    }

    in_maps = []
    for c in range(NCORES):
        m = dict(per_core[c])
        m["tab_a"], m["tab_p"], m["iota"] = tab_a, tab_p, iota
        # dst chunk features, transposed, padded to tile multiple
        xp = np.zeros((128, PT * 128), np.float32)
        xp[:, :PCHUNK] = inputs["x_paper"][c * PCHUNK:(c + 1) * PCHUNK].T
        xa = np.zeros((128, AT * 128), np.float32)
        xa[:, :ACHUNK] = inputs["x_author"][c * ACHUNK:(c + 1) * ACHUNK].T
        m["xT_p"], m["xT_a"] = xp, xa
        for k, v in wmap.items():
            m[k] = np.ascontiguousarray(v.astype(np.float32))
        for k, v in bmap.items():
            m[k] = np.ascontiguousarray(v.astype(np.float32))
        in_maps.append(m)

    res = run_bass_kernel_spmd(nc, in_maps, core_ids=list(range(NCORES)), trace=trace)

    out_paper = np.empty((N_PAPER, D), np.float32)
    out_author = np.empty((N_AUTHOR, D), np.float32)
    for c in range(NCORES):
        out_paper[c * PCHUNK:(c + 1) * PCHUNK] = res.results[c]["outT_p"][:, :PCHUNK].T
        out_author[c * ACHUNK:(c + 1) * ACHUNK] = res.results[c]["outT_a"][:, :ACHUNK].T
    return (out_paper, out_author), res


def kernel(**inputs):
    out, _ = _run(inputs, trace=False)
    return out


# revision 8
# speedup vs baseline: 2.6359x; 2.6359x over previous
"""Trainium2 Bass kernel for a 3-relation heterogeneous GraphConv encoder.

Graph: papers (50000) and authors (20000), D=H=128.
  out_paper  = mean_writes @ Wr_w.T + br_w + mean_cites @ Wr_c.T + br_c
               + x_paper @ (Wo_w + Wo_c).T
  out_author = mean_written @ Wr_n.T + br_n + x_author @ Wo_n.T
where mean_* is the per-destination mean over incoming edges of source features.

Sharding: destination nodes (and their incoming edges) are partitioned across
8 cores (papers 6250/core, authors 2500/core). Source feature tables are
replicated (bf16) and gathered per edge with dma_gather spread over all 4
SWDGE queues (each queue engages a different Q7 core pair). Edges are bucketed
by destination tile (128 dsts) on the host; per-core tiles are assigned to
SPMD slots in decreasing-load order so the shared per-slot run length (max
over cores) hugs each core's actual run length. Segment sums run on the
TensorEngine as one-hot matmuls (P[e, d] one-hot of in-tile dst, built on DVE
by comparing an iota row against per-edge dst ids), accumulated in PSUM fp32.
Means are divided by host-provided 1/clip(cnt,1), transposed on the PE, and
pushed through the (host-pre-transposed) relation weights; outputs are written
feature-major per slot and un-permuted/transposed on the host.
"""

import numpy as np
import ml_dtypes

import concourse.bacc as bacc
import concourse.mybir as mybir
import concourse.tile as tile
from concourse.bass_utils import run_bass_kernel_spmd
from concourse.library_config import mlp
from concourse.masks import make_identity

N_PAPER, N_AUTHOR, D = 50000, 20000, 128
NCORES = 8
PCHUNK, ACHUNK = N_PAPER // NCORES, N_AUTHOR // NCORES  # 6250, 2500
PT = (PCHUNK + 127) // 128   # 49 paper dst tiles / core
AT = (ACHUNK + 127) // 128   # 20 author dst tiles / core
HALF = 25000                 # paper gather table split (int16 index range)
G = 8192                     # gather chunk size (edges)
PB = 16                      # P-build batch (blocks)

BF16 = ml_dtypes.bfloat16

# stream key -> (n_dst_tiles, table key)
STREAMS = {
    "w":  (PT, "tab_a"),    # writes:   author -> paper
    "cA": (PT, "tab_pA"),   # cites:    paper(<25000) -> paper
    "cB": (PT, "tab_pB"),   # cites:    paper(>=25000) -> paper
    "nA": (AT, "tab_pA"),   # written:  paper(<25000) -> author
    "nB": (AT, "tab_pB"),   # written:  paper(>=25000) -> author
}
PAPER_STREAMS = ("w", "cA", "cB")
AUTHOR_STREAMS = ("nA", "nB")


def _bucket(src, dst, chunk, c):
    """Select edges into core c's dst chunk; return (src, tile_idx, q) sorted by tile."""
    lo, hi = c * chunk, (c + 1) * chunk
    sel = (dst >= lo) & (dst < hi)
    s = src[sel]
    loc = dst[sel] - lo
    t = loc >> 7
    order = np.argsort(t, kind="stable")
    return s[order], t[order], (loc & 127)[order]


def _prep_streams(inputs):
    """Host preprocessing: per-core bucketed/padded edge streams + counts.

    Returns (nbs, per_core, perms) where nbs[k] is a per-slot array of block
    counts (shared across cores), per_core[c] the input arrays, and perms[c] =
    (paper_perm, author_perm) mapping slot -> dst tile for output assembly.
    """
    ei = {
        "w": (np.asarray(inputs["ei_writes_src"]), np.asarray(inputs["ei_writes_dst"]), PCHUNK),
        "c": (np.asarray(inputs["ei_cites_src"]), np.asarray(inputs["ei_cites_dst"]), PCHUNK),
        "n": (np.asarray(inputs["ei_written_src"]), np.asarray(inputs["ei_written_dst"]), ACHUNK),
    }
    raw = {k: [] for k in STREAMS}
    cnts = {"p_w": [], "p_c": [], "a_n": []}
    for c in range(NCORES):
        for rel, (src, dst, chunk) in ei.items():
            nt = PT if rel in ("w", "c") else AT
            s, t, q = _bucket(src, dst, chunk, c)
            cnt = np.bincount(t * 128 + q, minlength=nt * 128).astype(np.float32)
            if rel == "w":
                raw["w"].append((s, t, q))
                cnts["p_w"].append(cnt)
            else:
                a = s < HALF
                kA, kB = ("cA", "cB") if rel == "c" else ("nA", "nB")
                raw[kA].append((s[a], t[a], q[a]))
                raw[kB].append((s[~a] - HALF, t[~a], q[~a]))
                cnts["p_c" if rel == "c" else "a_n"].append(cnt)

    # per (core, stream, tile) run lengths
    runs = {k: np.stack([np.bincount(raw[k][c][1], minlength=STREAMS[k][0])
                         for c in range(NCORES)])
            for k in STREAMS}

    # per-core slot permutation: slot s gets the s-th most loaded tile
    paper_tot = runs["w"] + runs["cA"] + runs["cB"]        # [NCORES, PT]
    author_tot = runs["nA"] + runs["nB"]                   # [NCORES, AT]
    pperm = np.argsort(-paper_tot, axis=1, kind="stable")  # [NCORES, PT]
    aperm = np.argsort(-author_tot, axis=1, kind="stable")

    # shared per-slot block counts
    nbs = {}
    for k in STREAMS:
        perm = pperm if k in PAPER_STREAMS else aperm
        slot_runs = np.take_along_axis(runs[k], perm, axis=1)  # [NCORES, nt]
        mx = np.maximum(slot_runs.max(axis=0), 1)
        nbs[k] = ((mx + 127) // 128).astype(np.int64)           # [nt]

    # build padded idx/q arrays per core, slot-ordered
    per_core = [dict() for _ in range(NCORES)]
    for k in STREAMS:
        nt = STREAMS[k][0]
        nb = nbs[k]
        L_s = nb * 128
        off_s = np.concatenate([[0], np.cumsum(L_s)[:-1]])
        S = int(L_s.sum())
        perm = pperm if k in PAPER_STREAMS else aperm
        for c in range(NCORES):
            s, t, q = raw[k][c]
            inv = np.empty(nt, np.int64)
            inv[perm[c]] = np.arange(nt)
            slot = inv[t]
            run = np.bincount(t, minlength=nt)
            off_in_run = np.arange(len(t)) - np.concatenate([[0], np.cumsum(run)[:-1]])[t]
            pos = off_s[slot] + off_in_run
            idx_flat = np.zeros(S, np.int16)
            q_flat = np.full(S, -1.0, np.float32)
            idx_flat[pos] = s.astype(np.int16)
            q_flat[pos] = q
            wrapped = idx_flat.reshape(-1, 16).T
            per_core[c][f"idx_{k}"] = np.ascontiguousarray(np.tile(wrapped, (8, 1)))
            per_core[c][f"q_{k}"] = np.ascontiguousarray(
                q_flat.reshape(-1, 128).T.astype(BF16))

    # inverse counts, slot-ordered; [d, s] = 1/clip(cnt[tile(s)*128+d], 1)
    for c in range(NCORES):
        for key, nt, name, perm in (("p_w", PT, "ic_pw", pperm), ("p_c", PT, "ic_pc", pperm),
                                    ("a_n", AT, "ic_an", aperm)):
            ic = (1.0 / np.clip(cnts[key][c], 1.0, None)).reshape(nt, 128)
            per_core[c][name] = np.ascontiguousarray(ic[perm[c]].T)

    perms = [(pperm[c], aperm[c]) for c in range(NCORES)]
    return nbs, per_core, perms


def _build_program(nbs, pt_lim=PT, at_lim=AT):
    nc = bacc.Bacc("TRN2", target_bir_lowering=False, num_swdge_queues=4)
    f32, bf16, i16 = mybir.dt.float32, mybir.dt.bfloat16, mybir.dt.int16

    S_k = {k: int(np.sum(nbs[k]) * 128) for k in STREAMS}
    off_k = {k: np.concatenate([[0], np.cumsum(nbs[k])[:-1]]).astype(np.int64)
             for k in STREAMS}

    tabs = {
        "tab_a": nc.dram_tensor("tab_a", [N_AUTHOR, D], bf16, kind="ExternalInput"),
        "tab_p": nc.dram_tensor("tab_p", [N_PAPER, D], bf16, kind="ExternalInput"),
    }
    idx_d, q_d = {}, {}
    for k in STREAMS:
        idx_d[k] = nc.dram_tensor(f"idx_{k}", [128, S_k[k] // 16], i16, kind="ExternalInput")
        q_d[k] = nc.dram_tensor(f"q_{k}", [128, S_k[k] // 128], bf16, kind="ExternalInput")
    ic_pw = nc.dram_tensor("ic_pw", [128, PT], f32, kind="ExternalInput")
    ic_pc = nc.dram_tensor("ic_pc", [128, PT], f32, kind="ExternalInput")
    ic_an = nc.dram_tensor("ic_an", [128, AT], f32, kind="ExternalInput")
    xT_p = nc.dram_tensor("xT_p", [128, PT * 128], f32, kind="ExternalInput")
    xT_a = nc.dram_tensor("xT_a", [128, AT * 128], f32, kind="ExternalInput")
    wd = {n: nc.dram_tensor(n, [128, 128], f32, kind="ExternalInput")
          for n in ("wrT_w", "wrT_c", "wrT_n", "woT_w", "woT_c", "woT_n")}
    brd = {n: nc.dram_tensor(n, [128, 1], f32, kind="ExternalInput")
           for n in ("br_w", "br_c", "br_n")}
    iota_d = nc.dram_tensor("iota", [128, PB * 128], bf16, kind="ExternalInput")
    outT_p = nc.dram_tensor("outT_p", [128, PT * 128], f32, kind="ExternalOutput")
    outT_a = nc.dram_tensor("outT_a", [128, AT * 128], f32, kind="ExternalOutput")

    def table_ap(k):
        t = STREAMS[k][1]
        if t == "tab_a":
            return tabs["tab_a"][:, :]
        if t == "tab_pA":
            return tabs["tab_p"][0:HALF, :]
        return tabs["tab_p"][HALF:, :]

    with tile.TileContext(nc) as tc:
        nc.gpsimd.load_library(mlp)
        import contextlib
        with contextlib.ExitStack() as ctx:
            const = ctx.enter_context(tc.tile_pool(name="const", bufs=1))
            idxp = ctx.enter_context(tc.tile_pool(name="idxp", bufs=6))
            chunkp = ctx.enter_context(tc.tile_pool(name="chunkp", bufs=6))
            xtp = ctx.enter_context(tc.tile_pool(name="xtp", bufs=3))
            pp = ctx.enter_context(tc.tile_pool(name="pp", bufs=4))
            mp = ctx.enter_context(tc.tile_pool(name="mp", bufs=3))
            op = ctx.enter_context(tc.tile_pool(name="op", bufs=2))
            ps_seg = ctx.enter_context(tc.tile_pool(name="ps_seg", bufs=4, space="PSUM"))
            ps_tr = ctx.enter_context(tc.tile_pool(name="ps_tr", bufs=2, space="PSUM"))
            ps_fin = ctx.enter_context(tc.tile_pool(name="ps_fin", bufs=2, space="PSUM"))

            # ---- constants ----
            iota_t = const.tile([128, PB * 128], bf16)
            nc.sync.dma_start(iota_t[:], iota_d[:])
            ic_t = {}
            for name, dram, nt in (("pw", ic_pw, PT), ("pc", ic_pc, PT), ("an", ic_an, AT)):
                t = const.tile([128, nt], f32, tag=f"ic_{name}")
                nc.sync.dma_start(t[:], dram[:])
                ic_t[name] = t
            w_t = {}
            for n in wd:
                t = const.tile([128, 128], f32, tag=n)
                nc.sync.dma_start(t[:], wd[n][:])
                w_t[n] = t
            br_t = {}
            for n in brd:
                t = const.tile([128, 1], f32, tag=n)
                nc.sync.dma_start(t[:], brd[n][:])
                br_t[n] = t
            ident = const.tile([128, 128], f32, tag="ident")
            make_identity(nc, ident[:])
            woT_p = const.tile([128, 128], f32, tag="woT_p")
            nc.vector.tensor_add(woT_p[:], w_t["woT_w"][:], w_t["woT_c"][:])
            br_p = const.tile([128, 1], f32, tag="br_p")
            nc.vector.tensor_add(br_p[:], br_t["br_w"][:], br_t["br_c"][:])
            q_t = {}
            for k in STREAMS:
                t = const.tile([128, S_k[k] // 128], bf16, tag=f"q_{k}")
                nc.sync.dma_start(t[:], q_d[k][:])
                q_t[k] = t

            # ---- gather machinery ----
            chunks = {}
            qrr = [0]

            def get_block(k, g):
                ci = g // (G // 128)
                if (k, ci) not in chunks:
                    sz = min(G, S_k[k] - ci * G)
                    it = idxp.tile([128, sz // 16], i16, tag="idx")
                    nc.sync.dma_start(it[:], idx_d[k][:, ci * (G // 16): ci * (G // 16) + sz // 16])
                    ct = chunkp.tile([128, sz // 128, 128], bf16, tag="chunk")
                    nc.gpsimd.dma_gather(ct[:], table_ap(k), it[:], sz, sz, D,
                                         single_packet=False, queue_num=qrr[0] % 4)
                    qrr[0] += 1
                    chunks[(k, ci)] = ct
                return chunks[(k, ci)][:, g % (G // 128), :]

            def build_p(k, s, t0, bw):
                pt_ = pp.tile([128, bw, 128], bf16, tag="p")
                sl = q_t[k][:, int(off_k[k][s]) + t0: int(off_k[k][s]) + t0 + bw]
                nc.vector.tensor_tensor(
                    out=pt_[:],
                    in0=iota_t[:, : bw * 128].rearrange("p (a b) -> p a b", b=128),
                    in1=sl[:, :, None].to_broadcast([128, bw, 128]),
                    op=mybir.AluOpType.is_equal,
                )
                return pt_

            def seg_accum(ps, specs):
                total = sum(int(nbs[k][s]) for k, s in specs)
                done = 0
                for k, s in specs:
                    nb = int(nbs[k][s])
                    for t0 in range(0, nb, PB):
                        bw = min(PB, nb - t0)
                        pt_ = build_p(k, s, t0, bw)
                        for b in range(bw):
                            g = int(off_k[k][s]) + t0 + b
                            nc.tensor.matmul(
                                ps[:],
                                lhsT=pt_[:, b, :],
                                rhs=get_block(k, g),
                                start=(done == 0),
                                stop=(done == total - 1),
                            )
                            done += 1

            def mean_T(ps, ic_tile, s):
                m = mp.tile([128, 128], f32, tag="mean")
                nc.vector.tensor_scalar_mul(m[:], ps[:], ic_tile[:, s:s + 1])
                ptr = ps_tr.tile([128, 128], f32, tag="tr", space="PSUM")
                nc.tensor.transpose(ptr[:], m[:], ident[:])
                mt = mp.tile([128, 128], f32, tag="meanT")
                nc.scalar.copy(mt[:], ptr[:])
                return mt

            # ---- paper dst slots ----
            stage = None
            for s in range(pt_lim):
                if s % 8 == 0:
                    gw = min(8, pt_lim - s)
                    stage = op.tile([128, gw * 128], f32, tag="stage_p")
                psw = ps_seg.tile([128, 128], f32, tag="seg", space="PSUM")
                seg_accum(psw, [("w", s)])
                psc = ps_seg.tile([128, 128], f32, tag="seg", space="PSUM")
                seg_accum(psc, [("cA", s), ("cB", s)])
                mtw = mean_T(psw, ic_t["pw"], s)
                mtc = mean_T(psc, ic_t["pc"], s)
                xt = xtp.tile([128, 128], f32, tag="xt")
                nc.sync.dma_start(xt[:], xT_p[:, s * 128:(s + 1) * 128])
                po = ps_fin.tile([128, 128], f32, tag="fin", space="PSUM")
                nc.tensor.matmul(po[:], lhsT=w_t["wrT_w"][:], rhs=mtw[:], start=True, stop=False)
                nc.tensor.matmul(po[:], lhsT=w_t["wrT_c"][:], rhs=mtc[:], start=False, stop=False)
                nc.tensor.matmul(po[:], lhsT=woT_p[:], rhs=xt[:], start=False, stop=True)
                nc.scalar.activation(stage[:, (s % 8) * 128:(s % 8) * 128 + 128], po[:],
                                     mybir.ActivationFunctionType.Identity, bias=br_p[:, :1])
                if s % 8 == 7 or s == pt_lim - 1:
                    s0 = (s // 8) * 8
                    nc.sync.dma_start(outT_p[:, s0 * 128:(s + 1) * 128], stage[:])

            # ---- author dst slots ----
            for s in range(at_lim):
                if s % 8 == 0:
                    gw = min(8, at_lim - s)
                    stage = op.tile([128, gw * 128], f32, tag="stage_a")
                psn = ps_seg.tile([128, 128], f32, tag="seg", space="PSUM")
                seg_accum(psn, [("nA", s), ("nB", s)])
                mtn = mean_T(psn, ic_t["an"], s)
                xt = xtp.tile([128, 128], f32, tag="xt")
                nc.sync.dma_start(xt[:], xT_a[:, s * 128:(s + 1) * 128])
                po = ps_fin.tile([128, 128], f32, tag="fin", space="PSUM")
                nc.tensor.matmul(po[:], lhsT=w_t["wrT_n"][:], rhs=mtn[:], start=True, stop=False)
                nc.tensor.matmul(po[:], lhsT=w_t["woT_n"][:], rhs=xt[:], start=False, stop=True)
                nc.scalar.activation(stage[:, (s % 8) * 128:(s % 8) * 128 + 128], po[:],
                                     mybir.ActivationFunctionType.Identity, bias=br_t["br_n"][:, :1])
                if s % 8 == 7 or s == at_lim - 1:
                    s0 = (s // 8) * 8
                    nc.sync.dma_start(outT_a[:, s0 * 128:(s + 1) * 128], stage[:])

    nc.compile()
    return nc


def _make_in_maps(inputs, per_core, perms):
    tab_a = inputs["x_author"].astype(BF16)
    tab_p = inputs["x_paper"].astype(BF16)
    iota = np.ascontiguousarray(
        np.broadcast_to(np.tile(np.arange(128, dtype=np.float32), PB), (128, PB * 128))
    ).astype(BF16)
    wmap = {
        "wrT_w": inputs["Wr_writes"].T, "wrT_c": inputs["Wr_cites"].T, "wrT_n": inputs["Wr_written"].T,
        "woT_w": inputs["Wo_writes"].T, "woT_c": inputs["Wo_cites"].T, "woT_n": inputs["Wo_written"].T,
    }
    bmap = {
        "br_w": inputs["br_writes"].reshape(128, 1), "br_c": inputs["br_cites"].reshape(128, 1),
        "br_n": inputs["br_written"].reshape(128, 1),
    }
    in_maps = []
    for c in range(NCORES):
        m = dict(per_core[c])
        m["tab_a"], m["tab_p"], m["iota"] = tab_a, tab_p, iota
        pperm, aperm = perms[c]
        xp_full = np.zeros((PT * 128, 128), np.float32)
        xp_full[:PCHUNK] = inputs["x_paper"][c * PCHUNK:(c + 1) * PCHUNK]
        xp = xp_full.reshape(PT, 128, 128)[pperm].reshape(PT * 128, 128).T
        xa_full = np.zeros((AT * 128, 128), np.float32)
        xa_full[:ACHUNK] = inputs["x_author"][c * ACHUNK:(c + 1) * ACHUNK]
        xa = xa_full.reshape(AT, 128, 128)[aperm].reshape(AT * 128, 128).T
        m["xT_p"], m["xT_a"] = np.ascontiguousarray(xp), np.ascontiguousarray(xa)
        for k2, v in wmap.items():
            m[k2] = np.ascontiguousarray(v.astype(np.float32))
        for k2, v in bmap.items():
            m[k2] = np.ascontiguousarray(v.astype(np.float32))
        in_maps.append(m)
    return in_maps


def _run(inputs, trace=False):
    inputs = {k: np.asarray(v) for k, v in inputs.items()}
    nbs, per_core, perms = _prep_streams(inputs)
    nc = _build_program(nbs)
    in_maps = _make_in_maps(inputs, per_core, perms)

    res = run_bass_kernel_spmd(nc, in_maps, core_ids=list(range(NCORES)), trace=trace)

    out_paper = np.empty((N_PAPER, D), np.float32)
    out_author = np.empty((N_AUTHOR, D), np.float32)
    for c in range(NCORES):
        pperm, aperm = perms[c]
        op_ = res.results[c]["outT_p"].T.reshape(PT, 128, 128)   # [slot, dst_in_tile, h]
        unp = np.empty_like(op_)
        unp[pperm] = op_
        out_paper[c * PCHUNK:(c + 1) * PCHUNK] = unp.reshape(PT * 128, 128)[:PCHUNK]
        oa_ = res.results[c]["outT_a"].T.reshape(AT, 128, 128)
        una = np.empty_like(oa_)
        una[aperm] = oa_
        out_author[c * ACHUNK:(c + 1) * ACHUNK] = una.reshape(AT * 128, 128)[:ACHUNK]
    return (out_paper, out_author), res


def kernel(**inputs):
    out, _ = _run(inputs, trace=False)
    return out


# revision 9
# speedup vs baseline: 4.2396x; 1.6084x over previous
"""Trainium2 Bass kernel for a 3-relation heterogeneous GraphConv encoder.

Graph: papers (50000) and authors (20000), D=H=128.
  out_paper  = mean_writes @ Wr_w.T + br_w + mean_cites @ Wr_c.T + br_c
               + x_paper @ (Wo_w + Wo_c).T
  out_author = mean_written @ Wr_n.T + br_n + x_author @ Wo_n.T
where mean_* is the per-destination mean over incoming edges of source features.

Sharding: destination nodes (and their incoming edges) are partitioned across
8 cores (papers 6250/core, authors 2500/core). Source feature tables are
replicated (bf16) and gathered per edge with dma_gather spread over all 4
SWDGE queues (each queue engages a different Q7 core pair). Edges are bucketed
by destination tile (128 dsts) on the host; per-core tiles are assigned to
SPMD slots in decreasing-load order so the shared per-slot run length (max
over cores) hugs each core's actual run length. Segment sums run on the
TensorEngine as one-hot matmuls (P[e, d] one-hot of in-tile dst, built on DVE
by comparing an iota row against per-edge dst ids), accumulated in PSUM fp32.
Means are divided by host-provided 1/clip(cnt,1), transposed on the PE, and
pushed through the (host-pre-transposed) relation weights; outputs are written
feature-major per slot and un-permuted/transposed on the host.
"""

import numpy as np
import ml_dtypes

import concourse.bacc as bacc
import concourse.mybir as mybir
import concourse.tile as tile
from concourse.bass_utils import run_bass_kernel_spmd
from concourse.library_config import mlp
from concourse.masks import make_identity

N_PAPER, N_AUTHOR, D = 50000, 20000, 128
NCORES = 8
PCHUNK, ACHUNK = N_PAPER // NCORES, N_AUTHOR // NCORES  # 6250, 2500
PT = (PCHUNK + 127) // 128   # 49 paper dst tiles / core
AT = (ACHUNK + 127) // 128   # 20 author dst tiles / core
HALF = 25000                 # paper gather table split (int16 index range)
G = 4096                     # gather chunk size (edges)
PB = 16                      # P-build batch (blocks)

BF16 = ml_dtypes.bfloat16

# stream key -> (n_dst_tiles, table key)
STREAMS = {
    "w":  (PT, "tab_a"),    # writes:   author -> paper
    "cA": (PT, "tab_pA"),   # cites:    paper(<25000) -> paper
    "cB": (PT, "tab_pB"),   # cites:    paper(>=25000) -> paper
    "nA": (AT, "tab_pA"),   # written:  paper(<25000) -> author
    "nB": (AT, "tab_pB"),   # written:  paper(>=25000) -> author
}
PAPER_STREAMS = ("w", "cA", "cB")
AUTHOR_STREAMS = ("nA", "nB")


def _bucket(src, dst, chunk, c):
    """Select edges into core c's dst chunk; return (src, tile_idx, q) sorted by tile."""
    lo, hi = c * chunk, (c + 1) * chunk
    sel = (dst >= lo) & (dst < hi)
    s = src[sel]
    loc = dst[sel] - lo
    t = loc >> 7
    order = np.argsort(t, kind="stable")
    return s[order], t[order], (loc & 127)[order]


def _prep_streams(inputs):
    """Host preprocessing: per-core bucketed/padded edge streams + counts.

    Returns (nbs, per_core, perms) where nbs[k] is a per-slot array of block
    counts (shared across cores), per_core[c] the input arrays, and perms[c] =
    (paper_perm, author_perm) mapping slot -> dst tile for output assembly.
    """
    ei = {
        "w": (np.asarray(inputs["ei_writes_src"]), np.asarray(inputs["ei_writes_dst"]), PCHUNK),
        "c": (np.asarray(inputs["ei_cites_src"]), np.asarray(inputs["ei_cites_dst"]), PCHUNK),
        "n": (np.asarray(inputs["ei_written_src"]), np.asarray(inputs["ei_written_dst"]), ACHUNK),
    }
    raw = {k: [] for k in STREAMS}
    cnts = {"p_w": [], "p_c": [], "a_n": []}
    for c in range(NCORES):
        for rel, (src, dst, chunk) in ei.items():
            nt = PT if rel in ("w", "c") else AT
            s, t, q = _bucket(src, dst, chunk, c)
            cnt = np.bincount(t * 128 + q, minlength=nt * 128).astype(np.float32)
            if rel == "w":
                raw["w"].append((s, t, q))
                cnts["p_w"].append(cnt)
            else:
                a = s < HALF
                kA, kB = ("cA", "cB") if rel == "c" else ("nA", "nB")
                raw[kA].append((s[a], t[a], q[a]))
                raw[kB].append((s[~a] - HALF, t[~a], q[~a]))
                cnts["p_c" if rel == "c" else "a_n"].append(cnt)

    # per (core, stream, tile) run lengths
    runs = {k: np.stack([np.bincount(raw[k][c][1], minlength=STREAMS[k][0])
                         for c in range(NCORES)])
            for k in STREAMS}

    # per-core slot permutation: slot s gets the s-th most loaded tile
    paper_tot = runs["w"] + runs["cA"] + runs["cB"]        # [NCORES, PT]
    author_tot = runs["nA"] + runs["nB"]                   # [NCORES, AT]
    pperm = np.argsort(-paper_tot, axis=1, kind="stable")  # [NCORES, PT]
    aperm = np.argsort(-author_tot, axis=1, kind="stable")

    # shared per-slot block counts
    nbs = {}
    for k in STREAMS:
        perm = pperm if k in PAPER_STREAMS else aperm
        slot_runs = np.take_along_axis(runs[k], perm, axis=1)  # [NCORES, nt]
        mx = np.maximum(slot_runs.max(axis=0), 1)
        nbs[k] = ((mx + 127) // 128).astype(np.int64)           # [nt]

    # build padded idx/q arrays per core, slot-ordered
    per_core = [dict() for _ in range(NCORES)]
    for k in STREAMS:
        nt = STREAMS[k][0]
        nb = nbs[k]
        L_s = nb * 128
        off_s = np.concatenate([[0], np.cumsum(L_s)[:-1]])
        S = int(L_s.sum())
        perm = pperm if k in PAPER_STREAMS else aperm
        for c in range(NCORES):
            s, t, q = raw[k][c]
            inv = np.empty(nt, np.int64)
            inv[perm[c]] = np.arange(nt)
            slot = inv[t]
            run = np.bincount(t, minlength=nt)
            off_in_run = np.arange(len(t)) - np.concatenate([[0], np.cumsum(run)[:-1]])[t]
            pos = off_s[slot] + off_in_run
            idx_flat = np.zeros(S, np.int16)
            q_flat = np.full(S, -1.0, np.float32)
            idx_flat[pos] = s.astype(np.int16)
            q_flat[pos] = q
            wrapped = idx_flat.reshape(-1, 16).T
            per_core[c][f"idx_{k}"] = np.ascontiguousarray(np.tile(wrapped, (8, 1)))
            per_core[c][f"q_{k}"] = np.ascontiguousarray(
                q_flat.reshape(-1, 128).T.astype(BF16))

    # inverse counts, slot-ordered; [d, s] = 1/clip(cnt[tile(s)*128+d], 1)
    for c in range(NCORES):
        for key, nt, name, perm in (("p_w", PT, "ic_pw", pperm), ("p_c", PT, "ic_pc", pperm),
                                    ("a_n", AT, "ic_an", aperm)):
            ic = (1.0 / np.clip(cnts[key][c], 1.0, None)).reshape(nt, 128)
            per_core[c][name] = np.ascontiguousarray(ic[perm[c]].T)

    perms = [(pperm[c], aperm[c]) for c in range(NCORES)]
    return nbs, per_core, perms


def _build_program(nbs, pt_lim=PT, at_lim=AT):
    nc = bacc.Bacc("TRN2", target_bir_lowering=False, num_swdge_queues=4)
    f32, bf16, i16 = mybir.dt.float32, mybir.dt.bfloat16, mybir.dt.int16

    S_k = {k: int(np.sum(nbs[k]) * 128) for k in STREAMS}
    off_k = {k: np.concatenate([[0], np.cumsum(nbs[k])[:-1]]).astype(np.int64)
             for k in STREAMS}

    tabs = {
        "tab_a": nc.dram_tensor("tab_a", [N_AUTHOR, D], bf16, kind="ExternalInput"),
        "tab_p": nc.dram_tensor("tab_p", [N_PAPER, D], bf16, kind="ExternalInput"),
    }
    idx_d, q_d = {}, {}
    for k in STREAMS:
        idx_d[k] = nc.dram_tensor(f"idx_{k}", [128, S_k[k] // 16], i16, kind="ExternalInput")
        q_d[k] = nc.dram_tensor(f"q_{k}", [128, S_k[k] // 128], bf16, kind="ExternalInput")
    ic_pw = nc.dram_tensor("ic_pw", [128, PT], f32, kind="ExternalInput")
    ic_pc = nc.dram_tensor("ic_pc", [128, PT], f32, kind="ExternalInput")
    ic_an = nc.dram_tensor("ic_an", [128, AT], f32, kind="ExternalInput")
    xT_p = nc.dram_tensor("xT_p", [128, PT * 128], f32, kind="ExternalInput")
    xT_a = nc.dram_tensor("xT_a", [128, AT * 128], f32, kind="ExternalInput")
    wd = {n: nc.dram_tensor(n, [128, 128], f32, kind="ExternalInput")
          for n in ("wrT_w", "wrT_c", "wrT_n", "woT_w", "woT_c", "woT_n")}
    brd = {n: nc.dram_tensor(n, [128, 1], f32, kind="ExternalInput")
           for n in ("br_w", "br_c", "br_n")}
    iota_d = nc.dram_tensor("iota", [128, PB * 128], bf16, kind="ExternalInput")
    outT_p = nc.dram_tensor("outT_p", [128, PT * 128], f32, kind="ExternalOutput")
    outT_a = nc.dram_tensor("outT_a", [128, AT * 128], f32, kind="ExternalOutput")

    def table_ap(k):
        t = STREAMS[k][1]
        if t == "tab_a":
            return tabs["tab_a"][:, :]
        if t == "tab_pA":
            return tabs["tab_p"][0:HALF, :]
        return tabs["tab_p"][HALF:, :]

    with tile.TileContext(nc) as tc:
        nc.gpsimd.load_library(mlp)
        import contextlib
        with contextlib.ExitStack() as ctx:
            const = ctx.enter_context(tc.tile_pool(name="const", bufs=1))
            idxp = ctx.enter_context(tc.tile_pool(name="idxp", bufs=10))
            chunkp = ctx.enter_context(tc.tile_pool(name="chunkp", bufs=10))
            xtp = ctx.enter_context(tc.tile_pool(name="xtp", bufs=3))
            pp = ctx.enter_context(tc.tile_pool(name="pp", bufs=4))
            mp = ctx.enter_context(tc.tile_pool(name="mp", bufs=3))
            op = ctx.enter_context(tc.tile_pool(name="op", bufs=2))
            ps_seg = ctx.enter_context(tc.tile_pool(name="ps_seg", bufs=4, space="PSUM"))
            ps_tr = ctx.enter_context(tc.tile_pool(name="ps_tr", bufs=2, space="PSUM"))
            ps_fin = ctx.enter_context(tc.tile_pool(name="ps_fin", bufs=2, space="PSUM"))

            # ---- constants ----
            iota_t = const.tile([128, PB * 128], bf16)
            nc.sync.dma_start(iota_t[:], iota_d[:])
            ic_t = {}
            for name, dram, nt in (("pw", ic_pw, PT), ("pc", ic_pc, PT), ("an", ic_an, AT)):
                t = const.tile([128, nt], f32, tag=f"ic_{name}")
                nc.sync.dma_start(t[:], dram[:])
                ic_t[name] = t
            w_t = {}
            for n in wd:
                t = const.tile([128, 128], f32, tag=n)
                nc.sync.dma_start(t[:], wd[n][:])
                w_t[n] = t
            br_t = {}
            for n in brd:
                t = const.tile([128, 1], f32, tag=n)
                nc.sync.dma_start(t[:], brd[n][:])
                br_t[n] = t
            ident = const.tile([128, 128], f32, tag="ident")
            make_identity(nc, ident[:])
            woT_p = const.tile([128, 128], f32, tag="woT_p")
            nc.vector.tensor_add(woT_p[:], w_t["woT_w"][:], w_t["woT_c"][:])
            br_p = const.tile([128, 1], f32, tag="br_p")
            nc.vector.tensor_add(br_p[:], br_t["br_w"][:], br_t["br_c"][:])
            q_t = {}
            for k in STREAMS:
                t = const.tile([128, S_k[k] // 128], bf16, tag=f"q_{k}")
                nc.sync.dma_start(t[:], q_d[k][:])
                q_t[k] = t

            # ---- gather machinery ----
            chunks = {}
            qrr = [0]

            def get_block(k, g):
                ci = g // (G // 128)
                if (k, ci) not in chunks:
                    sz = min(G, S_k[k] - ci * G)
                    it = idxp.tile([128, sz // 16], i16, tag="idx")
                    nc.sync.dma_start(it[:], idx_d[k][:, ci * (G // 16): ci * (G // 16) + sz // 16])
                    ct = chunkp.tile([128, sz // 128, 128], bf16, tag="chunk")
                    nc.gpsimd.dma_gather(ct[:], table_ap(k), it[:], sz, sz, D,
                                         single_packet=False, queue_num=qrr[0] % 4)
                    qrr[0] += 1
                    chunks[(k, ci)] = ct
                return chunks[(k, ci)][:, g % (G // 128), :]

            def build_p(k, s, t0, bw):
                pt_ = pp.tile([128, bw, 128], bf16, tag="p")
                sl = q_t[k][:, int(off_k[k][s]) + t0: int(off_k[k][s]) + t0 + bw]
                nc.vector.tensor_tensor(
                    out=pt_[:],
                    in0=iota_t[:, : bw * 128].rearrange("p (a b) -> p a b", b=128),
                    in1=sl[:, :, None].to_broadcast([128, bw, 128]),
                    op=mybir.AluOpType.is_equal,
                )
                return pt_

            def seg_accum(ps, specs):
                total = sum(int(nbs[k][s]) for k, s in specs)
                done = 0
                for k, s in specs:
                    nb = int(nbs[k][s])
                    for t0 in range(0, nb, PB):
                        bw = min(PB, nb - t0)
                        pt_ = build_p(k, s, t0, bw)
                        for b in range(bw):
                            g = int(off_k[k][s]) + t0 + b
                            nc.tensor.matmul(
                                ps[:],
                                lhsT=pt_[:, b, :],
                                rhs=get_block(k, g),
                                start=(done == 0),
                                stop=(done == total - 1),
                            )
                            done += 1

            def mean_T(ps, ic_tile, s):
                m = mp.tile([128, 128], f32, tag="mean")
                nc.vector.tensor_scalar_mul(m[:], ps[:], ic_tile[:, s:s + 1])
                ptr = ps_tr.tile([128, 128], f32, tag="tr", space="PSUM")
                nc.tensor.transpose(ptr[:], m[:], ident[:])
                mt = mp.tile([128, 128], f32, tag="meanT")
                nc.scalar.copy(mt[:], ptr[:])
                return mt

            # ---- paper dst slots ----
            stage = None
            for s in range(pt_lim):
                if s % 8 == 0:
                    gw = min(8, pt_lim - s)
                    stage = op.tile([128, gw * 128], f32, tag="stage_p")
                psw = ps_seg.tile([128, 128], f32, tag="seg", space="PSUM")
                seg_accum(psw, [("w", s)])
                psc = ps_seg.tile([128, 128], f32, tag="seg", space="PSUM")
                seg_accum(psc, [("cA", s), ("cB", s)])
                mtw = mean_T(psw, ic_t["pw"], s)
                mtc = mean_T(psc, ic_t["pc"], s)
                xt = xtp.tile([128, 128], f32, tag="xt")
                nc.sync.dma_start(xt[:], xT_p[:, s * 128:(s + 1) * 128])
                po = ps_fin.tile([128, 128], f32, tag="fin", space="PSUM")
                nc.tensor.matmul(po[:], lhsT=w_t["wrT_w"][:], rhs=mtw[:], start=True, stop=False)
                nc.tensor.matmul(po[:], lhsT=w_t["wrT_c"][:], rhs=mtc[:], start=False, stop=False)
                nc.tensor.matmul(po[:], lhsT=woT_p[:], rhs=xt[:], start=False, stop=True)
                nc.scalar.activation(stage[:, (s % 8) * 128:(s % 8) * 128 + 128], po[:],
                                     mybir.ActivationFunctionType.Identity, bias=br_p[:, :1])
                if s % 8 == 7 or s == pt_lim - 1:
                    s0 = (s // 8) * 8
                    nc.sync.dma_start(outT_p[:, s0 * 128:(s + 1) * 128], stage[:])

            # ---- author dst slots ----
            for s in range(at_lim):
                if s % 8 == 0:
                    gw = min(8, at_lim - s)
                    stage = op.tile([128, gw * 128], f32, tag="stage_a")
                psn = ps_seg.tile([128, 128], f32, tag="seg", space="PSUM")
                seg_accum(psn, [("nA", s), ("nB", s)])
                mtn = mean_T(psn, ic_t["an"], s)
                xt = xtp.tile([128, 128], f32, tag="xt")
                nc.sync.dma_start(xt[:], xT_a[:, s * 128:(s + 1) * 128])
                po = ps_fin.tile([128, 128], f32, tag="fin", space="PSUM")
                nc.tensor.matmul(po[:], lhsT=w_t["wrT_n"][:], rhs=mtn[:], start=True, stop=False)
                nc.tensor.matmul(po[:], lhsT=w_t["woT_n"][:], rhs=xt[:], start=False, stop=True)
                nc.scalar.activation(stage[:, (s % 8) * 128:(s % 8) * 128 + 128], po[:],
                                     mybir.ActivationFunctionType.Identity, bias=br_t["br_n"][:, :1])
                if s % 8 == 7 or s == at_lim - 1:
                    s0 = (s // 8) * 8
                    nc.sync.dma_start(outT_a[:, s0 * 128:(s + 1) * 128], stage[:])

    nc.compile()
    return nc


def _make_in_maps(inputs, per_core, perms):
    tab_a = inputs["x_author"].astype(BF16)
    tab_p = inputs["x_paper"].astype(BF16)
    iota = np.ascontiguousarray(
        np.broadcast_to(np.tile(np.arange(128, dtype=np.float32), PB), (128, PB * 128))
    ).astype(BF16)
    wmap = {
        "wrT_w": inputs["Wr_writes"].T, "wrT_c": inputs["Wr_cites"].T, "wrT_n": inputs["Wr_written"].T,
        "woT_w": inputs["Wo_writes"].T, "woT_c": inputs["Wo_cites"].T, "woT_n": inputs["Wo_written"].T,
    }
    bmap = {
        "br_w": inputs["br_writes"].reshape(128, 1), "br_c": inputs["br_cites"].reshape(128, 1),
        "br_n": inputs["br_written"].reshape(128, 1),
    }
    in_maps = []
    for c in range(NCORES):
        m = dict(per_core[c])
        m["tab_a"], m["tab_p"], m["iota"] = tab_a, tab_p, iota
        pperm, aperm = perms[c]
        xp_full = np.zeros((PT * 128, 128), np.float32)
        xp_full[:PCHUNK] = inputs["x_paper"][c * PCHUNK:(c + 1) * PCHUNK]
        xp = xp_full.reshape(PT, 128, 128)[pperm].reshape(PT * 128, 128).T
        xa_full = np.zeros((AT * 128, 128), np.float32)
        xa_full[:ACHUNK] = inputs["x_author"][c * ACHUNK:(c + 1) * ACHUNK]
        xa = xa_full.reshape(AT, 128, 128)[aperm].reshape(AT * 128, 128).T
        m["xT_p"], m["xT_a"] = np.ascontiguousarray(xp), np.ascontiguousarray(xa)
        for k2, v in wmap.items():
            m[k2] = np.ascontiguousarray(v.astype(np.float32))
        for k2, v in bmap.items():
            m[k2] = np.ascontiguousarray(v.astype(np.float32))
        in_maps.append(m)
    return in_maps


def _run(inputs, trace=False):
    inputs = {k: np.asarray(v) for k, v in inputs.items()}
    nbs, per_core, perms = _prep_streams(inputs)
    nc = _build_program(nbs)
    in_maps = _make_in_maps(inputs, per_core, perms)

    res = run_bass_kernel_spmd(nc, in_maps, core_ids=list(range(NCORES)), trace=trace)

    out_paper = np.empty((N_PAPER, D), np.float32)
    out_author = np.empty((N_AUTHOR, D), np.float32)
    for c in range(NCORES):
        pperm, aperm = perms[c]
        op_ = res.results[c]["outT_p"].T.reshape(PT, 128, 128)   # [slot, dst_in_tile, h]
        unp = np.empty_like(op_)
        unp[pperm] = op_
        out_paper[c * PCHUNK:(c + 1) * PCHUNK] = unp.reshape(PT * 128, 128)[:PCHUNK]
        oa_ = res.results[c]["outT_a"].T.reshape(AT, 128, 128)
        una = np.empty_like(oa_)
        una[aperm] = oa_
        out_author[c * ACHUNK:(c + 1) * ACHUNK] = una.reshape(AT * 128, 128)[:ACHUNK]
    return (out_paper, out_author), res


def kernel(**inputs):
    out, _ = _run(inputs, trace=False)
    return out
